# revision 1
# baseline (speedup 1.0000x reference)
import sys

for p in ("/opt/trn_rl_repo",):
    if p not in sys.path:
        sys.path.insert(0, p)

import numpy as np
import jax

# Persistent XLA compilation cache: run_bass_via_pjrt builds a fresh jit
# closure per call, so the in-memory pjit cache always misses (~0.4s/call).
# The HLO embeds the full compressed BIR (ant_bir in backend_config), so the
# disk cache key uniquely identifies the kernel.
jax.config.update("jax_compilation_cache_dir", "/tmp/jax_comp_cache")
jax.config.update("jax_persistent_cache_min_compile_time_secs", 0.0)
jax.config.update("jax_persistent_cache_min_entry_size_bytes", 0)
jax.config.update("jax_raise_persistent_cache_errors", False)

import concourse.bass as bass
import concourse.mybir as mybir
import concourse.tile as tile
from concourse import bacc, bass_utils

# Problem dims (hardcoded per contract)
B, S, DM, H, Dh = 2, 4096, 2048, 16, 128
NCORES = 8
SL = (B * S) // NCORES      # 1024 positions per core
P = 128
KT = DM // P                # 16 contraction tiles
HPC = H // NCORES           # 2 heads per core (weight shard)
WCOLS = 3 * HPC * Dh        # 768 shard columns (q|k|v)
GP = 8                      # positions per attention group (8*16 heads = 128)
GB = 4                      # groups per attention batch
NB = SL // (GP * GB)        # 32 batches per core
OC = Dh // 4 + 1            # output cols (fp32): 32 bitcast-uint8 data + 1 scale
WGR = DM + 2 * KT           # gathered weight rows: int8 data + packed fp16 scales
WROWS = WGR + 2 * P         # wt input rows: + bit-packed fp16 aux
INV_SQRT_D = 1.0 / float(np.sqrt(Dh))

_F16 = mybir.dt.float16
_NF16 = np.float16
AF = mybir.ActivationFunctionType
ALU = mybir.AluOpType


def _build_nc():
    """Fused per-core kernel: weight AllGather + QKV projections + per-position
    head-softmax attention. int8/fp16 inputs, uint8+scale packed fp32 output.

    Inputs (per core):
      xt [DM+2*KT, SL] int8 — rows [0:DM): x shard transposed, per-(pos,128-chunk)
                              symmetric int8 quant; rows [DM+2k, DM+2k+2): fp16
                              dequant scales xsc[k, :] bit-packed (x = q * xsc)
      wt [WROWS, WCOLS] int8 — rows [0:DM): W^T shard int8, per-(col,128-e-chunk)
                              quant: cols [0:256)=Wq^T(/sqrt(D)), [256:512)=Wk^T,
                              [512:768)=Wv^T; rows [DM:DM+2KT): fp16 dequant scale
                              bytes (2 rows per k); rows [DM+2KT:WROWS): fp16 aux
                              bytes: aux[P,768] = mask | vbb | bias
    Output:
      out [NB, P, GB, OC] fp32 — cols [0:32): 128 uint8 bit-packed:
                              round(y*127/amax + 127.5); col 32: scale
                              amax/softmax_denom. Host: (u-127.5)*scale/127.
    """
    nc = bacc.Bacc(None, target_bir_lowering=False, num_devices=NCORES)
    xt = nc.dram_tensor("xt", [DM + 2 * KT, SL], mybir.dt.int8, kind="ExternalInput")
    wt = nc.dram_tensor("wt", [WROWS, WCOLS], mybir.dt.int8, kind="ExternalInput")
    out = nc.dram_tensor("out", [NB, P, GB, OC], mybir.dt.float32,
                         kind="ExternalOutput")

    with tile.TileContext(nc) as tc:
        with tc.tile_pool(name="dram", bufs=1, space="DRAM") as dram, \
             tc.tile_pool(name="resident", bufs=1) as res:
            # ---- AllGather the weight shards (flat concat: chunk c = core c) ----
            wt_bounce = dram.tile([WGR, WCOLS], mybir.dt.int8)
            wfull = dram.tile([NCORES * WGR, WCOLS], mybir.dt.int8)
            # V in block layout: vdram[g, s*16+t, d] = V[8g+s, t*128+d]
            vdram = dram.tile([SL // GP, P, Dh], _F16)
            nc.sync.dma_start(wt_bounce[:], wt[0:WGR, :])
            nc.gpsimd.collective_compute(
                "AllGather",
                ALU.bypass,
                replica_groups=[list(range(NCORES))],
                ins=[wt_bounce[:].opt()],
                outs=[wfull[:].opt()],
            )

            # ---- Resident SBUF tensors ----
            xts = res.tile([P, KT, SL], _F16)      # x^T k-tiles (dequantized)
            # Q^T/K^T interleaved for the scores matmul: [d, group, s, head]
            qts = res.tile([P, SL // GP, GP, H], _F16)
            kts = res.tile([P, SL // GP, GP, H], _F16)
            vsb = res.tile([P, SL // P, DM], _F16) # V standard (s on partitions)
            aux16 = res.tile([P, WCOLS], _F16)
            mask32 = res.tile([P, P], mybir.dt.float32)
            bias32 = res.tile([P, 3 * KT], mybir.dt.float32)
            ones_sb = res.tile([P, 1], _F16)

            nc.sync.dma_start(aux16[:], wt[WGR:WROWS, :].bitcast(_F16))
            nc.vector.tensor_copy(mask32[:], aux16[:, 0:P])
            nc.vector.tensor_copy(bias32[:], aux16[:, 2 * P:2 * P + 3 * KT])
            nc.any.memset(ones_sb[:], 1.0)
            vbb16 = aux16[:, P:2 * P]

            def bias_ap(mat, t):
                c0 = mat * KT + t
                return bias32[:, c0:c0 + 1]

            # ---- dequantize x: xts[p,k,s] = xt_i8[128k+p, s] * xsc[k, s] ----
            with tc.tile_pool(name="deq", bufs=1) as dq, \
                 tc.tile_pool(name="xscr", bufs=2) as scp:
                xraw = dq.tile([P, KT, SL], mybir.dt.int8)
                nc.sync.dma_start(
                    xraw[:], xt[0:DM, :].rearrange("(k p) s -> p k s", p=P))
                for k in range(KT):
                    xscr = scp.tile([P, SL], _F16, tag="xscr")
                    nc.sync.dma_start(
                        xscr[:],
                        xt[DM + 2 * k:DM + 2 * k + 2, :].bitcast(_F16)[None]
                        .to_broadcast([P, 2, SL // 2]),
                    )
                    nc.vector.tensor_tensor(
                        xts[:, k, :], xraw[:, k, :], xscr[:], ALU.mult
                    )

            # ---- Q^T / K^T projections: out[m=f-tile(head), n=s] ----
            with tc.tile_pool(name="wstrip", bufs=3) as wpool, \
                 tc.tile_pool(name="psum_qk", bufs=2, space="PSUM") as pp:
                for mat, dst in ((0, qts), (1, kts)):
                    for t in range(H):
                        c, lc = t // HPC, mat * (HPC * Dh) + (t % HPC) * Dh
                        wq8 = wpool.tile([P, KT, P], mybir.dt.int8, tag="wq8")
                        nc.sync.dma_start(
                            wq8[:],
                            wfull[WGR * c:WGR * c + DM, lc:lc + P]
                            .rearrange("(k p) d -> p k d", p=P),
                        )
                        roff, boff = (2 * lc) // WCOLS, (2 * lc) % WCOLS
                        wscr = wpool.tile([P, KT, P], _F16, tag="wscr")
                        nc.sync.dma_start(
                            wscr[:],
                            wfull[WGR * c + DM:WGR * c + WGR, boff:boff + 2 * P]
                            .rearrange("(k two) b -> k two b", two=2)[:, roff, :]
                            .bitcast(_F16)[None].to_broadcast([P, KT, P]),
                        )
                        wstrip = wpool.tile([P, KT, P], _F16, tag="wstrip")
                        nc.vector.tensor_tensor(
                            wstrip[:], wq8[:], wscr[:], ALU.mult)
                        ps = pp.tile([P, SL], mybir.dt.float32, tag="ps")
                        for k in range(KT):
                            for j in range(2):
                                nc.tensor.matmul(
                                    ps[:, 512 * j:512 * (j + 1)],
                                    wstrip[:, k, :],
                                    xts[:, k, 512 * j:512 * (j + 1)],
                                    start=(k == 0),
                                    stop=(k == KT - 1),
                                )
                        nc.scalar.activation(
                            dst[:, :, :, t],
                            ps[:].rearrange("p (g s) -> p g s", s=GP),
                            AF.Identity,
                            bias=bias_ap(mat, t),
                        )

            # ---- V projection (standard layout): out[m=s-tile, n=f] ----
            with tc.tile_pool(name="wv", bufs=1) as wvpool, \
                 tc.tile_pool(name="psum_v", bufs=2, space="PSUM") as pv:
                for j in range(4):          # f-chunks of 512
                    wv8 = wvpool.tile([P, KT, 2, 256], mybir.dt.int8, tag="wv8")
                    wvsc = wvpool.tile([P, KT, 2, 256], _F16, tag="wvsc")
                    # chunk j covers heads 4j..4j+3 -> cores 2j, 2j+1, cols [512:768)
                    for cc in range(2):
                        base = WGR * (2 * j + cc)
                        nc.sync.dma_start(
                            wv8[:, :, cc, :],
                            wfull[base:base + DM, 2 * HPC * Dh:WCOLS]
                            .rearrange("(k p) d -> p k d", p=P),
                        )
                        # scale bytes for cols [512:768): row 2k+1, bytes [256:768)
                        nc.sync.dma_start(
                            wvsc[:, :, cc, :],
                            wfull[base + DM:base + WGR, 256:WCOLS]
                            .rearrange("(k two) b -> k two b", two=2)[:, 1, :]
                            .bitcast(_F16)[None].to_broadcast([P, KT, 256]),
                        )
                    wv = wvpool.tile([P, KT, 2, 256], _F16, tag="wv")
                    nc.vector.tensor_tensor(wv[:], wv8[:], wvsc[:], ALU.mult)
                    for mt in range(SL // P):
                        ps = pv.tile([P, 512], mybir.dt.float32, tag="psv")
                        for k in range(KT):
                            nc.tensor.matmul(
                                ps[:],
                                xts[:, k, P * mt:P * (mt + 1)],
                                wv[:, k, :, :],
                                start=(k == 0),
                                stop=(k == KT - 1),
                            )
                        nc.vector.tensor_copy(vsb[:, mt, 512 * j:512 * (j + 1)], ps[:])
            # scatter V to DRAM block layout: partition p=128mt+pp -> group
            # 16mt + pp//8, row (pp%8)*16+t, col d
            for mt in range(SL // P):
                nc.sync.dma_start(
                    vdram[16 * mt:16 * (mt + 1), :, :]
                    .rearrange("a (b t) d -> a b (t d)", t=H),
                    vsb[:, mt, :],
                )

            # ---- Attention: per 8-position group, 16x16 softmax over heads ----
            with tc.tile_pool(name="attn", bufs=3) as ap_, \
                 tc.tile_pool(name="psum_s", bufs=2, space="PSUM") as psp, \
                 tc.tile_pool(name="psum_o", bufs=2, space="PSUM") as pop, \
                 tc.tile_pool(name="psum_r", bufs=2, space="PSUM") as prp:
                for gb in range(NB):
                    vblk = ap_.tile([P, GB, Dh], _F16, tag="vblk")
                    nc.sync.dma_start(
                        vblk[:],
                        vdram[GB * gb:GB * (gb + 1), :, :].transpose([1, 0, 2]),
                    )
                    # V bias (zero in practice, kept for fidelity)
                    nc.vector.tensor_tensor(
                        vblk[:], vblk[:],
                        vbb16[:, None, :].to_broadcast([P, GB, Dh]),
                        ALU.add,
                    )
                    ps_s = psp.tile([P, GB, P], mybir.dt.float32, tag="ps_s")
                    for gi in range(GB):
                        g = gb * GB + gi
                        nc.tensor.matmul(
                            ps_s[:, gi, :],
                            kts[:, g],
                            qts[:, g],
                            start=True, stop=True,
                        )
                    nc.vector.tensor_tensor(
                        ps_s[:], ps_s[:],
                        mask32[:, None, :].to_broadcast([P, GB, P]),
                        ALU.add,
                    )
                    e_t = ap_.tile([P, GB, P], _F16, tag="e_t")
                    nc.scalar.activation(e_t[:], ps_s[:], AF.Exp)
                    ps_o = pop.tile([P, GB, Dh], mybir.dt.float32, tag="ps_o")
                    ps_r = prp.tile([P, GB], mybir.dt.float32, tag="ps_r")
                    for gi in range(GB):
                        nc.tensor.matmul(
                            ps_o[:, gi, :], e_t[:, gi, :], vblk[:, gi, :],
                            start=True, stop=True,
                        )
                        nc.tensor.matmul(
                            ps_r[:, gi:gi + 1], e_t[:, gi, :], ones_sb[:],
                            start=True, stop=True,
                        )
                    # per-(row, group) quantization; softmax denom folds into the
                    # shipped scale. DVE fp32->uint8 convert rounds to nearest:
                    # u = round(y*127/amax + 127.5), host decodes (u-127.5).
                    amax = ap_.tile([P, GB, 1], mybir.dt.float32, tag="amax")
                    nc.vector.tensor_reduce(
                        amax[:], ps_o[:], mybir.AxisListType.X, ALU.max,
                        apply_absolute_value=True,
                    )
                    nc.vector.tensor_scalar(
                        amax[:], amax[:], 1e-30, None, ALU.max,
                    )
                    rc = ap_.tile([P, GB], mybir.dt.float32, tag="rc")
                    nc.vector.reciprocal(rc[:], ps_r[:])
                    sc = ap_.tile([P, GB], mybir.dt.float32, tag="sc")
                    nc.vector.tensor_tensor(sc[:], amax[:, :, 0], rc[:], ALU.mult)
                    rq = ap_.tile([P, GB], mybir.dt.float32, tag="rq")
                    nc.vector.reciprocal(rq[:], amax[:, :, 0])
                    nc.vector.tensor_scalar(rq[:], rq[:], 127.0, None, ALU.mult)
                    ob = ap_.tile([P, GB, Dh], mybir.dt.uint8, tag="ob")
                    for gi in range(GB):
                        nc.vector.tensor_scalar(
                            ob[:, gi, :], ps_o[:, gi, :],
                            rq[:, gi:gi + 1], 127.5, ALU.mult, ALU.add,
                        )
                    nc.sync.dma_start(
                        out[gb][:, :, 0:Dh // 4].bitcast(mybir.dt.uint8), ob[:])
                    nc.sync.dma_start(out[gb][:, :, Dh // 4:OC], sc[:])
    nc.finalize()
    return nc


_NC_CACHE = None


def _get_nc():
    global _NC_CACHE
    if _NC_CACHE is None:
        _NC_CACHE = _build_nc()
    return _NC_CACHE


def build_in_maps(x, Wq, bq, Wk, bk, Wv, bv):
    """Host-side prep: quantize/transpose/shard/pack. Arrays are contiguous so
    the concatenate inside run_bass_kernel_spmd is a straight memcpy."""
    x = np.asarray(x, np.float32).reshape(NCORES, SL, DM)
    # per-(position, 128-chunk) symmetric int8 quant of x
    xc = x.reshape(NCORES, SL, KT, P)
    xsc_all = np.abs(xc).max(axis=3) / 127.0 + 1e-30      # [8, SL, KT]
    xq_all = np.round(xc / xsc_all[:, :, :, None]).astype(np.int8)
    xq_all = np.ascontiguousarray(
        xq_all.reshape(NCORES, SL, DM).transpose(0, 2, 1))      # [8, DM, SL]
    xsc_all = np.ascontiguousarray(
        xsc_all.astype(_NF16).transpose(0, 2, 1))               # [8, KT, SL]

    WqT = (np.asarray(Wq, np.float32) * INV_SQRT_D).T     # [e, f]
    WkT = np.asarray(Wk, np.float32).T
    WvT = np.asarray(Wv, np.float32).T
    WTall = np.concatenate([WqT, WkT, WvT], axis=1)       # [DM, 3*DM]
    # per-(col, 128-e-chunk) int8 quant of W^T
    wch = np.ascontiguousarray(WTall).reshape(KT, P, 3 * DM)
    wsc_all = np.abs(wch).max(axis=1) / 127.0 + 1e-30     # [KT, 3*DM]
    wq_all = np.round(wch / wsc_all[:, None, :]).astype(np.int8).reshape(DM, 3 * DM)
    wsc_all = wsc_all.astype(_NF16)

    bias = np.stack([
        np.asarray(bq, np.float32) * INV_SQRT_D,
        np.asarray(bk, np.float32),
        np.asarray(bv, np.float32),
    ]).astype(_NF16)

    aux = np.zeros((P, WCOLS), _NF16)
    blk = np.arange(P) // 16
    aux[:, 0:P] = np.where(blk[:, None] == blk[None, :], 0.0, -1e4)
    aux[:, P:2 * P] = np.asarray(bv, np.float32).reshape(H, Dh)[np.arange(P) % 16]
    # bias cols: 2P + mat*16 + t holds bias[mat, 128t+p]
    aux[:, 2 * P:2 * P + 3 * KT] = (
        bias.reshape(3, KT, P).transpose(2, 0, 1).reshape(P, 3 * KT))

    aux_rows = np.ascontiguousarray(aux).view(np.int8).reshape(2 * P, WCOLS)
    in_maps = []
    for c in range(NCORES):
        cols = np.r_[HPC * Dh * c:HPC * Dh * (c + 1),
                     DM + HPC * Dh * c:DM + HPC * Dh * (c + 1),
                     2 * DM + HPC * Dh * c:2 * DM + HPC * Dh * (c + 1)]
        wsc_rows = (np.ascontiguousarray(wsc_all[:, cols])
                    .view(np.int8).reshape(2 * KT, WCOLS))
        wt = np.concatenate([wq_all[:, cols], wsc_rows, aux_rows], axis=0)
        # pack fp16 scales as 2 int8 rows per k-tile below the x data
        xsc_rows = xsc_all[c].view(np.int8).reshape(2 * KT, SL)
        xt = np.concatenate([xq_all[c], xsc_rows], axis=0)      # [DM+2KT, SL]
        in_maps.append({"xt": xt, "wt": wt})
    return in_maps


def kernel(x, Wq, bq, Wk, bk, Wv, bv):
    nc = _get_nc()
    in_maps = build_in_maps(x, Wq, bq, Wk, bk, Wv, bv)
    res = bass_utils.run_bass_kernel_spmd(nc, in_maps, core_ids=list(range(NCORES)))

    # out[c][gb, s*16+h, g, :32] bit-packs 128 uint8; col 32 is the scale.
    # value = F[b, h, gb*32+g*8+s, d] for b = c//4, positions offset (c%4)*SL.
    # Reference returns F.reshape(B, S, H*D) with F = [B, H, S, D].
    F = np.empty((B, H, S, Dh), np.float32)
    for c in range(NCORES):
        b, sc = c // (NCORES // B), c % (NCORES // B)
        r = np.ascontiguousarray(res.results[c]["out"])
        u = r[:, :, :, 0:Dh // 4].view(np.uint8)                # [NB, P, GB, 128]
        scl = r[:, :, :, Dh // 4]                               # [NB, P, GB]
        oc = (u.astype(np.float32) - 127.5) * (scl[:, :, :, None] / 127.0)
        F[b, :, SL * sc:SL * (sc + 1), :] = (
            oc.reshape(NB, GP, H, GB, Dh).transpose(2, 0, 3, 1, 4).reshape(H, SL, Dh)
        )
    return F.reshape(B, S, H * Dh)

